# revision 46
# baseline (speedup 1.0000x reference)
"""Trainium2 Bass kernel for DTWFeatures.

Problem: x (64,3,1024), patts (32,3,32) -> out (64,32,1024)
  dist[b,p,l,t] = sqrt(max(|x[b,:,t]-patts[p,:,l]|^2, eps))
  DP:  D[l,t] = dist[l,t] + min(D[l-1,t], w*D[l,t-1], w*D[l-1,t-1])
  out[b,p,t] = D[L-1,t]

Strategy (8 cores, data-parallel over batch, 8 batches/core):
  * Rescale E[l,t] = D[l,t]*w^-(t-SHIFT) which removes w from the recurrence:
        E[l,t] = dist'[l,t] + min(E[l,t-1], E[l-1,t], E[l-1,t-1])
    with dist'[l,t] = dist[l,t]*w^-(t-SHIFT).  SHIFT=512 keeps all
    magnitudes within fp32 range (w^-2(t-SHIFT) in [1e-32, 8.7e31]).
  * Per row l this is a first-order recurrence solved by ONE DVE
    tensor_tensor_scan (op0=min, op1=add):
        state_t = min(c_t, state_{t-1}) + dist'_t,  c_t = min(E[l-1,t], E[l-1,t-1])
  * dist'^2 is produced directly by TensorE as a K=17 matmul:
        out[(b4,p), t] = sum_k lhsT[k,(b,p)] * rhs[k,t]
    with lhsT rows = block-diag -2*patts (12), per-b x2-indicators (4),
    p2+eps (1) and rhs rows = x*w2inv (12), x2*w2inv (4), w2inv (1).
    ScalarE (ACT) then applies sqrt PSUM->SBUF.
  * 256 pairs/core = 2 groups of 128 partitions -> two independent
    (window-min -> scan) chains per row that interleave on DVE.  TensorE,
    ScalarE and the DMAs run well ahead; DVE is the bottleneck engine
    (~150us busy; TensorTensor and scan are fp32 1x ops and GPSIMD cannot
    execute TensorTensor at all on trn2 codegen).
"""

import sys

if "/opt/trn_rl_repo" not in sys.path:
    sys.path.insert(0, "/opt/trn_rl_repo")

import numpy as np

NB, ND, NP, NL, NT = 64, 3, 32, 32, 1024   # batch, xdim, n_patts, l_patts, T
NCORES = 8
BPC = NB // NCORES                     # 8 batches per core
RHO = 0.1
W = RHO ** (1.0 / NL)
SHIFT = 512.0
EPS = 2e-5
INF = 1.0e30
K = 17                                 # matmul contraction rows

SEGS = 1         # scan segments per DP row

_CACHE = {}


def _tables():
    if "tables" not in _CACHE:
        t = np.arange(NT, dtype=np.float64)
        w2inv = (W ** (-2.0 * (t - SHIFT))).astype(np.float32)
        wpos = (W ** (t - SHIFT)).astype(np.float32)
        W2INV17 = np.ascontiguousarray(np.tile(w2inv[None, :], (K, 1)))
        WPOS2 = np.ascontiguousarray(np.tile(wpos[None, None, :], (128, 2, 1)))
        INDIC = np.zeros((4, NL, 128), np.float32)
        for bq in range(4):
            INDIC[bq, :, bq * 32 : (bq + 1) * 32] = 1.0
        _CACHE["tables"] = (W2INV17, WPOS2, np.ascontiguousarray(INDIC))
    return _CACHE["tables"]


def _build(debug=False):
    key = ("nc", debug)
    if key in _CACHE:
        return _CACHE[key]

    from contextlib import ExitStack

    import concourse.bass as bass  # noqa: F401
    import concourse.tile as tile
    from concourse import bacc, mybir

    f32 = mybir.dt.float32
    AOT = mybir.AluOpType

    nc = bacc.Bacc(None, target_bir_lowering=False)
    x8 = nc.dram_tensor("x8", [BPC, ND, NT], f32, kind="ExternalInput")
    patts_d = nc.dram_tensor("patts_in", [NP, ND, NL], f32, kind="ExternalInput")
    w2inv_d = nc.dram_tensor("w2inv17", [K, NT], f32, kind="ExternalInput")
    wpos_d = nc.dram_tensor("wpos2", [128, 2, NT], f32, kind="ExternalInput")
    indic_d = nc.dram_tensor("indic", [4, NL, 128], f32, kind="ExternalInput")
    out_d = nc.dram_tensor("out8", [BPC, NP, NT], f32, kind="ExternalOutput")
    if debug:
        dbg_lhsT = nc.dram_tensor("dbg_lhsT", [K, NL, 128], f32, kind="ExternalOutput")
        dbg_xw = nc.dram_tensor("dbg_xw", [2, K, NT], f32, kind="ExternalOutput")
        dbg_d = nc.dram_tensor("dbg_d", [2, 128, 2, NT], f32, kind="ExternalOutput")
        dbg_E = nc.dram_tensor("dbg_E", [4, 128, 2, NT + 1], f32, kind="ExternalOutput")

    with tile.TileContext(nc) as tc:
        with ExitStack() as ctx:
            persist = ctx.enter_context(tc.tile_pool(name="persist", bufs=1))
            dist_pool = ctx.enter_context(tc.tile_pool(name="dist", bufs=4))
            c_pool = ctx.enter_context(tc.tile_pool(name="cmin", bufs=4))
            psum_pool = ctx.enter_context(
                tc.tile_pool(name="psum", bufs=3, space="PSUM")
            )
            outp = ctx.enter_context(tc.tile_pool(name="outp", bufs=1))

            lhsT = persist.tile([K, NL, 128], f32, name="lhsT")
            w2inv = persist.tile([K, NT], f32, name="w2inv")
            wpos = persist.tile([128, 2, NT], f32, name="wpos")
            inf2 = persist.tile([128, NT], f32, name="inf2")
            E0 = persist.tile([128, 2, NT + 1], f32, name="E0")
            E1 = persist.tile([128, 2, NT + 1], f32, name="E1")
            E = [E0, E1]

            xg0 = persist.tile([12, NT], f32, name="xg0")
            xg1 = persist.tile([12, NT], f32, name="xg1")
            xa0 = persist.tile([4, 3 * NT], f32, name="xa0")
            xa1 = persist.tile([4, 3 * NT], f32, name="xa1")
            x20 = persist.tile([4, NT], f32, name="x20")
            x21 = persist.tile([4, NT], f32, name="x21")
            xw0 = persist.tile([K, NT], f32, name="xw0")
            xw1 = persist.tile([K, NT], f32, name="xw1")
            xg, xa, x2, xw = [xg0, xg1], [xa0, xa1], [x20, x21], [xw0, xw1]

            pp = persist.tile([NP, ND, NL], f32, name="pp")      # (p, d, l) natural
            ppsq = persist.tile([NP, ND, NL], f32, name="ppsq")
            p2e = persist.tile([NP, NL], f32, name="p2e")        # (p, l)
            p2eT = persist.tile([NL, NP], f32, name="p2eT")      # (l, p)
            ppT = persist.tile([NL, ND, NP], f32, name="ppT")    # (l, d, p)

            # ---------------- input DMAs ----------------
            nc.sync.dma_start(w2inv[:], w2inv_d[:])
            nc.sync.dma_start(wpos[:], wpos_d[:])
            for h in range(2):
                bs = h * 4
                nc.sync.dma_start(
                    xg[h][:], x8[bs : bs + 4].rearrange("b d t -> (b d) t")
                )
                nc.sync.dma_start(
                    xa[h][:], x8[bs : bs + 4].rearrange("b d t -> b (d t)")
                )
            nc.sync.dma_start(pp[:], patts_d[:])

            # ---------------- lhsT build ----------------
            nc.gpsimd.memset(lhsT[:], 0.0)
            # transpose patts blocks to (l, d, p) so p is contiguous for placement;
            # scale by -2 for the |x-p|^2 cross term
            for d in range(ND):
                nc.vector.transpose(ppT[:, d, :], pp[:, d, :])
            nc.vector.tensor_scalar_mul(ppT[:], ppT[:], -2.0)
            # p2 + eps row (row 16)
            nc.scalar.square(ppsq[:], pp[:])
            nc.vector.tensor_tensor(
                p2e[:], ppsq[:, 0, :], ppsq[:, 1, :], op=AOT.add
            )
            nc.vector.tensor_tensor(p2e[:], p2e[:], ppsq[:, 2, :], op=AOT.add)
            nc.vector.tensor_scalar_add(p2e[:], p2e[:], EPS)
            nc.vector.transpose(p2eT[:], p2e[:])
            nc.sync.dma_start(lhsT[12:16, :, :], indic_d[:])
            for bq in range(4):
                bs = bq * 32
                nc.sync.dma_start(lhsT[16:17, :, bs : bs + 32], p2eT[:])
                for d in range(ND):
                    nc.sync.dma_start(
                        lhsT[bq * 3 + d : bq * 3 + d + 1, :, bs : bs + 32],
                        ppT[:, d, :],
                    )

            # ---------------- rhs (xw) build ----------------
            for h in range(2):
                nc.scalar.square(xa[h][:], xa[h][:])
                nc.vector.tensor_tensor(
                    x2[h][:], xa[h][:, 0:NT], xa[h][:, NT : 2 * NT], op=AOT.add
                )
                nc.vector.tensor_tensor(
                    x2[h][:], x2[h][:], xa[h][:, 2 * NT : 3 * NT], op=AOT.add
                )
                # x2 * w2inv at partitions 0..3, then DMA into xw rows 12..15
                nc.vector.tensor_tensor(
                    x2[h][:], x2[h][:], w2inv[0:4, :], op=AOT.mult
                )
                nc.vector.tensor_tensor(
                    xw[h][0:12, :], xg[h][:], w2inv[0:12, :], op=AOT.mult
                )
                nc.sync.dma_start(xw[h][12:16, :], x2[h][:])
                nc.sync.dma_start(xw[h][16:17, :], w2inv_d[16:17, :])

            # ---------------- DP state init ----------------
            nc.vector.memset(inf2[:], INF)
            nc.vector.memset(E0[:, :, 0:1], INF)
            nc.vector.memset(E1[:, :, 0:1], INF)

            # ---------------- main loop over DP rows ----------------
            for j in range(NL):
                d3 = dist_pool.tile([128, 2, NT], f32, name="d3")
                for hh in range(2):
                    ps = psum_pool.tile([128, NT], f32, name="ps")
                    nc.tensor.matmul(
                        ps[:, 0:512],
                        lhsT[:, j, :],
                        xw[hh][:, 0:512],
                        start=True,
                        stop=True,
                    )
                    nc.tensor.matmul(
                        ps[:, 512:1024],
                        lhsT[:, j, :],
                        xw[hh][:, 512:1024],
                        start=True,
                        stop=True,
                    )
                    nc.scalar.sqrt(d3[:, hh, :], ps[:])
                if debug and j < 2:
                    nc.sync.dma_start(dbg_d[j], d3[:])

                Ecur, Eprev = E[j % 2], E[(j + 1) % 2]
                HS = NT // SEGS  # scan segment size
                segs = [(s * HS, (s + 1) * HS) for s in range(SEGS)]
                if j == 0:
                    for hh in range(2):
                        for s0, s1 in segs:
                            nc.vector.tensor_tensor_scan(
                                out=Ecur[:, hh, s0 + 1 : s1 + 1],
                                data0=inf2[:, s0:s1],
                                data1=d3[:, hh, s0:s1],
                                initial=0.0 if s0 == 0 else Ecur[:, hh, s0 : s0 + 1],
                                op0=AOT.min,
                                op1=AOT.add,
                            )
                    if debug:
                        nc.sync.dma_start(dbg_E[0], Ecur[:])
                        nc.sync.dma_start(dbg_lhsT[:], lhsT[:])
                        nc.sync.dma_start(dbg_xw[0], xw[0][:])
                        nc.sync.dma_start(dbg_xw[1], xw[1][:])
                else:
                    c3 = c_pool.tile([128, 2, NT], f32, name="c3")
                    for hh in range(2):
                        # window-min + scan both on DVE (the only engine that
                        # can run TensorTensor/scan); the two h-chains
                        # interleave to keep DVE busy
                        eng = nc.vector
                        for s0, s1 in segs:
                            eng.tensor_tensor(
                                c3[:, hh : hh + 1, s0:s1],
                                Eprev[:, hh : hh + 1, s0 + 1 : s1 + 1],
                                Eprev[:, hh : hh + 1, s0:s1],
                                op=AOT.min,
                            )
                            nc.vector.tensor_tensor_scan(
                                out=Ecur[:, hh, s0 + 1 : s1 + 1],
                                data0=c3[:, hh, s0:s1],
                                data1=d3[:, hh, s0:s1],
                                initial=INF if s0 == 0 else Ecur[:, hh, s0 : s0 + 1],
                                op0=AOT.min,
                                op1=AOT.add,
                            )
                    if debug and 1 <= j <= 3:
                        nc.sync.dma_start(dbg_E[j], Ecur[:])

            # ---------------- output ----------------
            Elast = E[(NL - 1) % 2]
            oth = outp.tile([128, 2, NT], f32, name="oth")
            nc.vector.tensor_tensor(
                oth[:], Elast[:, :, 1 : NT + 1], wpos[:], op=AOT.mult
            )
            of = out_d.rearrange("b p t -> (b p) t")
            for hh in range(2):
                nc.sync.dma_start(of[hh * 128 : (hh + 1) * 128, :], oth[:, hh, :])

    nc.compile()
    _CACHE[key] = nc
    return nc


def _in_maps(x, patts):
    W2INV17, WPOS2, INDIC = _tables()
    x = np.ascontiguousarray(np.asarray(x, dtype=np.float32))
    patts = np.ascontiguousarray(np.asarray(patts, dtype=np.float32))
    maps = []
    for c in range(NCORES):
        maps.append(
            {
                "x8": np.ascontiguousarray(x[c * BPC : (c + 1) * BPC]),
                "patts_in": patts,
                "w2inv17": W2INV17,
                "wpos2": WPOS2,
                "indic": INDIC,
            }
        )
    return maps


def kernel(x, patts):
    nc = _build()
    from concourse.bass_utils import run_bass_kernel_spmd

    res = run_bass_kernel_spmd(
        nc, _in_maps(x, patts), core_ids=list(range(NCORES))
    )
    _CACHE["last_results"] = res
    out = np.concatenate([r["out8"] for r in res.results], axis=0)
    return out.astype(np.float32)
